# revision 3
# baseline (speedup 1.0000x reference)
"""DA-RNN (dual-stage attention RNN) Trainium2 Bass kernel — v2.

Key changes vs baseline (661us):
  * Encoder input-attention computed BATCH-major: softmax runs along the
    free dim, so the denominator comes free via activation accum_out, the
    reciprocal is a [128,1] op (~100ns vs 880ns single-lane), and
    normalize+x-mult fuse into one scalar_tensor_tensor. One PE transpose
    brings xin back feature-major for the gate matmuls.
  * LSTM cells in "2x domain": states C=2c, H=2h; sigmoid(x)=(tanh(x/2)+1)/2
    folds into fused stt ops ((t+1)*y), 0.5 absorbed into consumer weights.
  * Decoder logits DMA'd straight from PSUM (no evacuation copy), issued
    from the idle GpSimd engine; ctx weighted-sum folds 1/Z into the first
    multiply.

Layouts (per core, b=256 = 2 chains of 128):
  encT  [128h, 50l, 256b] bf16   enc states 2h, feature-major (matmul lhsT)
  encBh [128b, 2c, 50l, 128h]    enc states batch-major (ctx weighted sum)
"""

import numpy as np
import ml_dtypes

import concourse.bacc as bacc
import concourse.tile as tile
import concourse.mybir as mybir
from concourse.bass_utils import run_bass_kernel_spmd

F32 = mybir.dt.float32
BF16 = mybir.dt.bfloat16
AF = mybir.ActivationFunctionType
OP = mybir.AluOpType

L, NOUT, F, B, H = 50, 3, 64, 2048, 128
NC = 8
BPC = B // NC          # 256 batch per core
CH = 2                 # encoder chains (batch halves of 128)
BH = BPC // CH         # 128

bf16 = ml_dtypes.bfloat16

# PyTorch gate order in weights is (i, f, g, o); we reorder to (f, i, o, g)
# so sigmoid gates (f,i,o) are contiguous and tanh gate (g) is last.
GATE_PERM = [1, 0, 3, 2]  # rows of 4xH blocks: f, i, o, g


def _gate_rows(w, g):
    """rows of gate g (in f,i,o,g order) from a (4H, X) matrix."""
    src = GATE_PERM[g]
    return w[src * H:(src + 1) * H]


def prep_inputs(inputs):
    """Host-side prep: returns (shared weight arrays, per-core input arrays)."""
    f32 = np.float32
    x = np.asarray(inputs["x"], f32)            # [B, L, F]

    shared = {}
    # encoder attention dense: e_bm[b,:] = [x_t,1] @ w1b_t + (2h) @ w2h_t
    aw = np.asarray(inputs["attn_w"], f32)      # [L, F+H, F]
    ab = np.asarray(inputs["attn_b"], f32)      # [L, F]
    w1b = np.zeros((F + 1, L, F), f32)
    w1b[:F] = aw[:, :F, :].transpose(1, 0, 2)
    w1b[F] = ab                                  # bias row
    shared["w1b"] = w1b.astype(bf16)
    shared["w2h"] = np.ascontiguousarray(
        0.5 * aw[:, F:, :].transpose(1, 0, 2)).astype(bf16)   # [128, L, 64]

    # encoder LSTM. 65-row Wih lhsT: row 64 carries the combined bias.
    # whhT consumes H=2h so scale 0.5.
    wih = np.asarray(inputs["enc_Wih"], f32)    # [4H, F]
    whh = np.asarray(inputs["enc_Whh"], f32)    # [4H, H]
    bias = np.asarray(inputs["enc_bih"], f32) + np.asarray(inputs["enc_bhh"], f32)
    # g-gate (slot 3) rows are pre-doubled so tanh(pg*0.5) yields tanh(g)
    # for it while the sigmoid gates get tanh(gate/2) — one ACT op for all 4.
    wih65 = np.zeros((F + 1, 4, H), f32)
    whhT = np.zeros((H, 4, H), f32)
    for g in range(4):
        gg = 2.0 if g == 3 else 1.0
        wih65[:F, g, :] = gg * _gate_rows(wih, g).T
        wih65[F, g, :] = gg * _gate_rows(bias[:, None], g)[:, 0]
        whhT[:, g, :] = gg * 0.5 * _gate_rows(whh, g).T
    shared["wih65"] = wih65.astype(bf16)
    shared["whhT"] = whhT.astype(bf16)

    # decoder attention (enc states arrive as 2h -> scale 0.5)
    ddw = np.asarray(inputs["dd_w"], f32)       # [NOUT, 2H, H]
    shared["ddw1"] = np.ascontiguousarray(
        0.5 * ddw[:, :H, :].transpose(1, 0, 2)).astype(bf16)  # [128, NOUT, 128]
    shared["ddw2"] = np.ascontiguousarray(
        0.5 * ddw[:, H:, :].transpose(1, 0, 2)).astype(bf16)
    shared["ddb"] = np.ascontiguousarray(np.asarray(inputs["dd_b"], f32).T)  # [128, NOUT]
    dlw = np.asarray(inputs["dl_w"], f32)[:, :, 0].T                         # [NOUT,128]->T
    shared["dlw"] = np.ascontiguousarray(
        np.repeat(dlw[:, :, None], 32, axis=2)).astype(bf16)                 # [128, NOUT, 32]
    shared["dlb"] = np.asarray(inputs["dl_b"], f32)[:, 0]                    # [NOUT]

    # decoder LSTM: dec_in = [ctx, dec_out]; dec_out == h_de so those input
    # columns merge with Whh. ctx arrives 2x (built from 2h enc), h_de is 2x.
    dwih = np.asarray(inputs["dec_Wih"], f32)   # [4H, 2H]
    dwhh = np.asarray(inputs["dec_Whh"], f32)   # [4H, H]
    dbias = np.asarray(inputs["dec_bih"], f32) + np.asarray(inputs["dec_bhh"], f32)
    wdic = np.zeros((H, 4, H), f32)
    wdoh = np.zeros((H, 4, H), f32)
    dbias_r = np.zeros((1, 4, H), f32)
    for g in range(4):
        gg = 2.0 if g == 3 else 1.0
        wdic[:, g, :] = gg * 0.5 * _gate_rows(dwih[:, :H], g).T
        wdoh[:, g, :] = gg * 0.5 * (_gate_rows(dwih[:, H:], g) + _gate_rows(dwhh, g)).T
        dbias_r[0, g, :] = gg * _gate_rows(dbias[:, None], g)[:, 0]
    shared["wdic"] = wdic.astype(bf16)
    shared["wdoh"] = wdoh.astype(bf16)
    shared["dbias"] = dbias_r.astype(bf16)

    # heads (h_de arrives 2x -> fcw scale 0.5)
    shared["fcw"] = np.ascontiguousarray(
        0.5 * np.asarray(inputs["fc_w"], f32).transpose(1, 0, 2)).astype(bf16)  # [128,NOUT,64]
    shared["fcb"] = np.ascontiguousarray(np.asarray(inputs["fc_b"], f32).T)     # [64, NOUT]
    shared["outw"] = np.ascontiguousarray(
        np.asarray(inputs["out_w"], f32)[:, :, 0].T).astype(bf16)               # [64, NOUT]
    shared["outb"] = np.asarray(inputs["out_b"], f32)[:, 0]                     # [NOUT]

    shared["ident"] = np.eye(BH, dtype=np.float32).astype(bf16)                 # [128,128]

    per_core = []
    for c in range(NC):
        xc = x[c * BPC:(c + 1) * BPC]           # [256, L, F]
        xT65 = np.ones((F + 1, L, BPC), f32)
        xT65[:F] = xc.transpose(2, 1, 0)
        xbm = np.ascontiguousarray(
            xc.reshape(CH, BH, L, F).transpose(1, 0, 2, 3))  # [128, 2, L, 64]
        per_core.append({"xT65": xT65.astype(bf16), "xbm": xbm.astype(bf16)})
    return shared, per_core


def build_program():
    nc = bacc.Bacc("TRN2", target_bir_lowering=False, debug=False, num_devices=NC)

    dram = {}

    def din(name, shape, dt):
        dram[name] = nc.dram_tensor(name, shape, dt, kind="ExternalInput").ap()
        return dram[name]

    din("xT65", (F + 1, L, BPC), BF16)
    din("xbm", (BH, CH, L, F), BF16)
    din("w1b", (F + 1, L, F), BF16)
    din("w2h", (H, L, F), BF16)
    din("wih65", (F + 1, 4, H), BF16)
    din("whhT", (H, 4, H), BF16)
    din("ddw1", (H, NOUT, H), BF16)
    din("ddw2", (H, NOUT, H), BF16)
    din("ddb", (H, NOUT), F32)
    din("dlw", (H, NOUT, 32), BF16)
    din("wdic", (H, 4, H), BF16)
    din("wdoh", (H, 4, H), BF16)
    din("dbias", (1, 4, H), BF16)
    din("fcw", (H, NOUT, F), BF16)
    din("fcb", (F, NOUT), F32)
    din("outw", (F, NOUT), BF16)
    din("ident", (BH, BH), BF16)
    y_out = nc.dram_tensor("y", (NOUT, BPC), F32, kind="ExternalOutput").ap()
    dlb_sc = build_program.scalars["dlb"]
    outb_sc = build_program.scalars["outb"]

    with tile.TileContext(nc) as tc:
        _body(nc, tc, dram, y_out, dlb_sc, outb_sc)
    nc.compile()
    return nc, list(dram.keys())


build_program.scalars = {"dlb": [0.0] * NOUT, "outb": [0.0] * NOUT}


def _body(nc, tc, dram, y_out, dlb_sc, outb_sc):
    import contextlib
    ctx = contextlib.ExitStack()
    with ctx:
        singles = ctx.enter_context(tc.tile_pool(name="singles", bufs=1))

        # ---- persistent SBUF tensors ----
        def load(name, shape, dt):
            t = singles.tile(list(shape), dt, tag=name)
            nc.sync.dma_start(out=t, in_=dram[name])
            return t

        xT65 = load("xT65", (F + 1, L, BPC), BF16)
        xbm = load("xbm", (BH, CH, L, F), BF16)
        w1b = load("w1b", (F + 1, L, F), BF16)
        w2h = load("w2h", (H, L, F), BF16)
        wih65 = load("wih65", (F + 1, 4, H), BF16)
        whhT = load("whhT", (H, 4, H), BF16)
        ddw1 = load("ddw1", (H, NOUT, H), BF16)
        ddw2 = load("ddw2", (H, NOUT, H), BF16)
        ddb = load("ddb", (H, NOUT), F32)
        dlw = load("dlw", (H, NOUT, 32), BF16)
        wdic = load("wdic", (H, 4, H), BF16)
        wdoh = load("wdoh", (H, 4, H), BF16)
        dbias = load("dbias", (1, 4, H), BF16)
        fcw = load("fcw", (H, NOUT, F), BF16)
        fcb = load("fcb", (F, NOUT), F32)
        outw = load("outw", (F, NOUT), BF16)
        ident = load("ident", (BH, BH), BF16)

        encT = singles.tile([H, L, BPC], BF16, tag="encT")
        encBh = singles.tile([BH, CH, L, H], BF16, tag="encBh")
        xin65 = singles.tile([F + 1, CH, 2, BH], BF16, tag="xin65")
        cstate = singles.tile([H, CH, BH], F32, tag="cstate")   # C = 2c
        onesrow = singles.tile([1, BPC], BF16, tag="onesrow")
        hdeT = singles.tile([H, BPC], BF16, tag="hdeT")         # 2*h_de
        dcst = singles.tile([H, BPC], F32, tag="dcst")          # 2*c_de
        ySB = singles.tile([1, NOUT, BPC], F32, tag="ySB")

        dlbT = singles.tile([64, NOUT], F32, tag="dlbT")
        outbT = singles.tile([1, NOUT], F32, tag="outbT")
        for i in range(NOUT):
            nc.vector.memset(dlbT[:, i:i + 1], float(dlb_sc[i]))
            nc.vector.memset(outbT[:, i:i + 1], float(outb_sc[i]) * 0.5)

        nc.vector.memset(xin65[F:F + 1, :, :, :], 1.0)
        nc.vector.memset(cstate, 0.0)
        nc.vector.memset(onesrow, 1.0)
        nc.vector.memset(dcst, 0.0)

        # ================= encoder =================
        with tc.tile_pool(name="psE", bufs=3, space="PSUM") as psE, \
             tc.tile_pool(name="psT", bufs=2, space="PSUM") as psT, \
             tc.tile_pool(name="psG", bufs=2, space="PSUM") as psG, \
             tc.tile_pool(name="enc_sb", bufs=3) as sb:

            for t in range(L):
                for c in range(CH):
                    bs = slice(c * BH, (c + 1) * BH)
                    h_prev = encT[:, t - 1, bs] if t > 0 else None

                    # LSTM recurrent part first (depends only on h_prev)
                    pg = psG.tile([H, 4, BH], F32, tag="pg")
                    if t > 0:
                        for g in range(4):
                            nc.tensor.matmul(pg[:, g, :], whhT[:, g, :], h_prev,
                                             start=True, stop=False)

                    # input attention, batch-major:
                    #   e_bm[b,f] = [x_t,1]@w1b_t + (2h)@w2h_t
                    pe = psE.tile([BH, F], F32, tag="pe")
                    nc.tensor.matmul(pe, xT65[:, t, bs], w1b[:, t, :],
                                     start=True, stop=(t == 0))
                    if t > 0:
                        nc.tensor.matmul(pe, h_prev, w2h[:, t, :],
                                         start=False, stop=True)
                    ebm = sb.tile([BH, F], BF16, tag="ebm")
                    nc.scalar.activation(ebm, pe, AF.Tanh)
                    ubm = sb.tile([BH, F], BF16, tag="ubm")
                    zb = sb.tile([BH, 1], F32, tag="zb")
                    nc.scalar.activation(ubm, ebm, AF.Exp, accum_out=zb)
                    rz = sb.tile([BH, 1], F32, tag="rz")
                    nc.vector.reciprocal(rz, zb)
                    # xin = softmax(e) * x_t, still batch-major
                    xinb = sb.tile([BH, F], BF16, tag="xinb")
                    nc.vector.scalar_tensor_tensor(
                        xinb, ubm, rz, xbm[:, c, t, :], op0=OP.mult, op1=OP.mult)
                    # transpose to feature-major for the gate matmul
                    pxT = psT.tile([F, BH], BF16, tag="pxT")
                    nc.tensor.transpose(pxT, xinb, ident)
                    xslot = xin65[:, c, t % 2, :]
                    nc.vector.tensor_copy(xslot[0:F, :], pxT)

                    # gates += Wih65 @ [xin; 1]
                    for g in range(4):
                        nc.tensor.matmul(pg[:, g, :], wih65[:, g, :], xslot,
                                         start=(t == 0), stop=True)

                    # LSTM cell in 2x domain; f,i,o give tanh(gate/2), the
                    # g slot gives tanh(g) (its weights are pre-doubled).
                    tfio = sb.tile([H, 3, BH], BF16, tag="tfio")
                    nc.scalar.activation(tfio, pg[:, 0:3, :], AF.Tanh, scale=0.5)
                    tg = sb.tile([H, BH], BF16, tag="tg")
                    nc.scalar.activation(tg, pg[:, 3, :], AF.Tanh, scale=0.5)
                    mi = sb.tile([H, BH], F32, tag="mi")
                    nc.vector.scalar_tensor_tensor(
                        mi, tfio[:, 1, :], 1.0, tg, op0=OP.add, op1=OP.mult)
                    mf = sb.tile([H, BH], F32, tag="mf")
                    nc.vector.scalar_tensor_tensor(
                        mf, tfio[:, 0, :], 1.0, cstate[:, c, :],
                        op0=OP.add, op1=OP.mult)
                    nc.vector.scalar_tensor_tensor(
                        cstate[:, c, :], mf, 0.5, mi, op0=OP.mult, op1=OP.add)
                    tcn = sb.tile([H, BH], BF16, tag="tcn")
                    nc.scalar.activation(tcn, cstate[:, c, :], AF.Tanh, scale=0.5)
                    nc.vector.scalar_tensor_tensor(
                        encT[:, t, bs], tfio[:, 2, :], 1.0, tcn,
                        op0=OP.add, op1=OP.mult)
                    # batch-major copy for decoder context sums
                    nc.sync.dma_start_transpose(encBh[:, c, t, :], encT[:, t, bs])

        # ================= decoder =================
        CHK = 4  # l per chunk
        nchunks = (L + CHK - 1) // CHK
        for i in range(NOUT):
            with tc.tile_pool(name="psDD", bufs=2, space="PSUM") as psDD, \
                 tc.tile_pool(name="psL", bufs=2, space="PSUM") as psL, \
                 tc.tile_pool(name="psX", bufs=1, space="PSUM") as psX, \
                 tc.tile_pool(name="dec_sb", bufs=3) as sb, \
                 tc.tile_pool(name="ctx_sb", bufs=1) as csb:
                logitsL = sb.tile([64, BPC], F32, tag="logitsL")
                nc.vector.memset(logitsL, 0.0)
                for k in range(nchunks):
                    nl = min(CHK, L - k * CHK)
                    pdd = psDD.tile([H, CHK, BPC], F32, tag="pdd")
                    # matmul out is limited to one psum bank (512 fp32):
                    # emit the chunk as 2-l (N=512) pieces.
                    for j0 in range(0, nl, 2):
                        j1 = min(j0 + 2, nl)
                        nc.tensor.matmul(pdd[:, j0:j1, :], ddw1[:, i, :],
                                         encT[:, k * CHK + j0:k * CHK + j1, :],
                                         start=True, stop=(i == 0))
                        if i > 0:
                            nc.tensor.matmul(
                                pdd[:, j0:j1, :], ddw2[:, i, :],
                                hdeT[:, None, :].broadcast_to([H, j1 - j0, BPC]),
                                start=False, stop=True)
                    e2c = sb.tile([H, CHK, BPC], BF16, tag="e2c")
                    nc.scalar.activation(e2c[:, 0:nl, :], pdd[:, 0:nl, :], AF.Tanh,
                                         bias=ddb[:, i:i + 1])
                    # logits: each l fills a 32-row col-group of the psum
                    # block (replicated dl_w columns); DMA-gather 4 rows
                    # straight out of PSUM from the idle GpSimd engine.
                    pl = psL.tile([H, BPC], F32, tag="pl")
                    for j in range(nl):
                        nc.tensor.matmul(pl[32 * j:32 * (j + 1), :],
                                         dlw[:, i, :], e2c[:, j, :],
                                         start=True, stop=True,
                                         tile_position=(0, 32 * j))
                    lsc = sb.tile([H, BPC], F32, tag="lsc")
                    if k % 2 == 0:
                        nc.vector.tensor_copy(lsc, pl)
                    else:
                        nc.scalar.copy(lsc, pl)
                    geng = nc.sync if k % 2 == 0 else nc.gpsimd
                    geng.dma_start(out=logitsL[k * CHK:k * CHK + nl, :],
                                   in_=lsc[0:32 * nl:32, :])
                expL = sb.tile([64, BPC], BF16, tag="expL")
                nc.vector.memset(expL, 0.0)
                nc.scalar.activation(expL[0:L, :], logitsL[0:L, :], AF.Exp,
                                     bias=dlbT[0:L, i:i + 1])
                expB = sb.tile([BH, CH, 64], BF16, tag="expB")
                for hh in range(CH):
                    pxb = psX.tile([BH, 64], BF16, tag="pxb")
                    nc.tensor.transpose(pxb, expL[:, hh * BH:(hh + 1) * BH],
                                        ident[0:64, 0:64])
                    if hh == 0:
                        nc.scalar.copy(expB[:, hh, :], pxb)
                    else:
                        nc.vector.tensor_copy(expB[:, hh, :], pxb)
                z = sb.tile([BH, CH], F32, tag="z")
                nc.vector.tensor_reduce(z, expB[:, :, 0:L], axis=mybir.AxisListType.X,
                                        op=OP.add)
                rz = sb.tile([BH, CH], F32, tag="rz")
                nc.vector.reciprocal(rz, z)

                # ctx = sum_l alpha * enc — tree reduction over l in encBh's
                # native [b, c, l, h] layout (h innermost, alpha broadcast
                # along h); 1/Z folded into the first multiply.
                prod = csb.tile([BH, CH, L, H], BF16, tag="prod")
                for hh in range(CH):
                    nc.vector.scalar_tensor_tensor(
                        prod[:, hh], expB[:, hh, 0:L, None].broadcast_to([BH, L, H]),
                        rz[:, hh:hh + 1], encBh[:, hh, :, :],
                        op0=OP.mult, op1=OP.mult)
                s25 = csb.tile([BH, CH, 25, H], BF16, tag="s25")
                nc.vector.tensor_tensor(s25, prod[:, :, 0:25, :],
                                        prod[:, :, 25:50, :], op=OP.add)
                s5 = sb.tile([BH, CH, 5, H], BF16, tag="s5")
                v25 = s25.rearrange("b c (lo li) h -> b c lo li h", lo=5)
                nc.vector.tensor_tensor(s5, v25[:, :, 0, :, :], v25[:, :, 1, :, :],
                                        op=OP.add)
                for j in (2, 3, 4):
                    nc.vector.tensor_tensor(s5, s5, v25[:, :, j, :, :], op=OP.add)
                cn = sb.tile([BH, CH, H], BF16, tag="cn")
                ctxr = sb.tile([BH, CH, H], F32, tag="ctxr")
                nc.vector.tensor_tensor(ctxr, s5[:, :, 0, :], s5[:, :, 1, :],
                                        op=OP.add)
                for j in (2, 3):
                    nc.vector.tensor_tensor(ctxr, ctxr, s5[:, :, j, :], op=OP.add)
                nc.vector.tensor_tensor(cn, ctxr, s5[:, :, 4, :], op=OP.add)
                ctxT = sb.tile([H, BPC], BF16, tag="ctxT")
                for hh in range(CH):
                    pcT = psX.tile([H, BH], BF16, tag="pcT")
                    nc.tensor.transpose(pcT, cn[:, hh, :], ident)
                    if hh == 0:
                        nc.scalar.copy(ctxT[:, hh * BH:(hh + 1) * BH], pcT)
                    else:
                        nc.vector.tensor_copy(ctxT[:, hh * BH:(hh + 1) * BH], pcT)

            # decoder LSTM + heads (2x domain like the encoder)
            with tc.tile_pool(name="psDG", bufs=1, space="PSUM") as psDG, \
                 tc.tile_pool(name="psY", bufs=1, space="PSUM") as psY, \
                 tc.tile_pool(name="dlstm_sb", bufs=2) as sb:
                pg = psDG.tile([H, 4, BPC], F32, tag="pdg")
                for g in range(4):
                    nc.tensor.matmul(pg[:, g, :], wdic[:, g, :], ctxT,
                                     start=True, stop=False)
                    if i > 0:
                        nc.tensor.matmul(pg[:, g, :], wdoh[:, g, :], hdeT,
                                         start=False, stop=False)
                    nc.tensor.matmul(pg[:, g, :], dbias[:, g, :], onesrow,
                                     start=False, stop=True)
                # slots f,i,o hold tanh(gate/2); slot 3 tanh(g) (pre-doubled).
                # Two ACTs, one per PSUM bank of the 2-bank pg tile.
                tall = sb.tile([H, 4, BPC], BF16, tag="dtall")
                nc.scalar.activation(tall[:, 0:2, :], pg[:, 0:2, :], AF.Tanh,
                                     scale=0.5)
                nc.scalar.activation(tall[:, 2:4, :], pg[:, 2:4, :], AF.Tanh,
                                     scale=0.5)
                mi = sb.tile([H, BPC], F32, tag="dmi")
                nc.vector.scalar_tensor_tensor(
                    mi, tall[:, 1, :], 1.0, tall[:, 3, :],
                    op0=OP.add, op1=OP.mult)
                mf = sb.tile([H, BPC], F32, tag="dmf")
                nc.vector.scalar_tensor_tensor(
                    mf, tall[:, 0, :], 1.0, dcst, op0=OP.add, op1=OP.mult)
                nc.vector.scalar_tensor_tensor(
                    dcst, mf, 0.5, mi, op0=OP.mult, op1=OP.add)
                tcn = sb.tile([H, BPC], BF16, tag="dtcn")
                nc.scalar.activation(tcn, dcst, AF.Tanh, scale=0.5)
                nc.vector.scalar_tensor_tensor(
                    hdeT, tall[:, 2, :], 1.0, tcn, op0=OP.add, op1=OP.mult)

                py1 = psY.tile([F, BPC], F32, tag="py1")
                nc.tensor.matmul(py1, fcw[:, i, :], hdeT, start=True, stop=True)
                y1 = sb.tile([F, BPC], BF16, tag="y1")
                nc.scalar.activation(y1, py1, AF.Tanh, bias=fcb[:, i:i + 1])
                py2 = psY.tile([1, BPC], F32, tag="py2")
                nc.tensor.matmul(py2, outw[:, i:i + 1], y1, start=True, stop=True)
                yt = sb.tile([1, BPC], F32, tag="yt")
                nc.scalar.activation(yt, py2, AF.Tanh, scale=0.5,
                                     bias=outbT[:, i:i + 1])
                nc.vector.tensor_scalar(ySB[:, i, :], yt, 0.5, 0.5,
                                        op0=OP.mult, op1=OP.add)

        nc.sync.dma_start(out=y_out, in_=ySB)


_CACHE = {}


def kernel(**inputs):
    return _run(inputs, trace=False)[0]


def kernel_profiled(**inputs):
    """Returns (output, BassKernelResults) with NTFF trace/exec time."""
    return _run(inputs, trace=True)


def _run(inputs, trace=False):
    shared, per_core = prep_inputs(inputs)
    key = (float(shared["dlb"][0]), float(shared["outb"][0]),
           float(shared["dlb"][-1]), float(shared["outb"][-1]))
    if key not in _CACHE:
        build_program.scalars = {"dlb": shared["dlb"].tolist(),
                                 "outb": shared["outb"].tolist()}
        _CACHE[key] = build_program()
    nc, names = _CACHE[key]
    in_maps = []
    for c in range(NC):
        m = dict(shared)
        m.pop("dlb"), m.pop("outb")
        m.update(per_core[c])
        in_maps.append({k: np.ascontiguousarray(v) for k, v in m.items()})
    res = run_bass_kernel_spmd(nc, in_maps, core_ids=list(range(NC)), trace=trace)
    outs = [res.results[c]["y"].T for c in range(NC)]   # [BPC, NOUT] each
    return np.concatenate(outs, axis=0).astype(np.float32), res


if __name__ == "__main__":
    pass
